# revision 3
# baseline (speedup 1.0000x reference)
"""Trainium2 Bass kernel for AttentionFact:
    scores = einsum('bsh,ch->bcs', hidden, querys)
    factor = softmax(scores, axis=2)
    out    = einsum('bcs,bsh->bch', factor, hidden).reshape(B, C*H)

Shapes: B=16, S=4096, H=1024, C=64, fp32.

Strategy (v5): data-parallel over batch, 2 batches per core, querys
replicated.  Host pre-casts hidden to fp16 (halves HBM traffic) and
pre-transposes querys into a duplicated [128, 8, 128] bank.

PE work is column-tiled throughout (tile_position (0,0)/(0,64)):
  - hT transposes: each 128x128 block is split into two concurrent
    64-col transposes -> ~2x faster transpose pass.
  - scores: s-tiles processed in PAIRS; member 0 accumulates into
    PSUM rows 0:64, member 1 into rows 64:128 (duplicated qT weights).
  - pooling: column groups split H (h 0:512 rows 0:64, h 512:1024
    rows 64:128) with identical fT weights.

Softmax: flash-style over two per-batch halves, with DEFERRED exp:
scores stay in PSUM until the half's max is known, then exp uses
bias = -M_half directly (no per-tile beta fold pass).

Optionally some hT groups are produced by DRAM-direct xbar DMA
transpose instead of the PE (XBAR_JPS).
"""

import numpy as np

import concourse.bass as bass
import concourse.mybir as mybir
import concourse.tile as tile
from concourse import bacc
from concourse.bass_utils import run_bass_kernel_spmd

B, S, H, C = 16, 4096, 1024, 64
NCORES = 8
BPC = B // NCORES          # batches per core
ST = 8                     # s-tiles per batch (512 rows each)
SQ = 4                     # 128-row subtiles per s-tile
HJ = H // 128              # h-chunks (8)
NPAIR = ST // 2            # s-tile pairs per batch
LOOKAHEAD = 4              # tile loads kept in flight ahead of compute

F32 = mybir.dt.float32
F16 = mybir.dt.float16

# (b, st, jp) triples whose hT group (2 chunks) comes from DRAM via the
# xbar DMA transpose instead of PE transposes (jp in 0..3)
XBAR_JPS = set()


def build_nc():
    nc = bacc.Bacc("TRN2", target_bir_lowering=False, debug=False)
    hidden = nc.declare_dram_parameter("hidden", [BPC, S, H], F16, isOutput=False)
    # qT2[k, j, c] = qT2[k, j, 64+c] = querys[c, j*128+k]
    qT2 = nc.declare_dram_parameter("qT2", [128, HJ, 128], F16, isOutput=False)
    ident = nc.declare_dram_parameter("ident", [128, 128], F16, isOutput=False)
    out = nc.declare_dram_parameter("out", [BPC, C, H], F32, isOutput=True)

    with tile.TileContext(nc) as tc:
        with (
            tc.tile_pool(name="const", bufs=1) as const_pool,
            tc.tile_pool(name="nat", bufs=2 * ST) as nat_pool,
            tc.tile_pool(name="hT", bufs=12) as hT_pool,
            tc.tile_pool(name="expp", bufs=2) as exp_pool,
            tc.tile_pool(name="fT", bufs=2) as fT_pool,
            tc.tile_pool(name="stats", bufs=2) as stats_pool,
            tc.tile_pool(name="outp", bufs=2) as out_pool,
            tc.tile_pool(name="psT", bufs=2, space="PSUM") as psT_pool,
            tc.tile_pool(name="psS", bufs=4, space="PSUM") as psS_pool,
            tc.tile_pool(name="psR", bufs=2, space="PSUM") as psR_pool,
        ):
            ident_sb = const_pool.tile([128, 128], F16, tag="ident")
            nc.sync.dma_start(out=ident_sb[:], in_=ident[:])
            qT_sb = const_pool.tile([128, HJ, 128], F16, tag="qT2")
            nc.sync.dma_start(out=qT_sb[:], in_=qT2[:])

            nat_tiles = {}
            hT_sets = {}
            ps_pair = {}    # pair p -> psS tile holding scores of (2p, 2p+1)
            exp_tiles = {}
            st_stats = {}   # b -> stats tile [64, 32] f32 laid out below
            ps_half = {}    # (b, half) -> psum accumulator [128, 512]

            # stats tile columns: 0:8 negm (-tile max), 8:16 rowsum,
            # 24:26 negM half, 26:28 gamma half, 28:30 S halves
            NEGM, RS, NEGM_H, GAM_H, SH = 0, 8, 24, 26, 28

            issued = []

            def load_tile(b, st):
                nat_t = nat_pool.tile([128, SQ, H], F16, tag="nat",
                                      name=f"nat{b}_{st}")
                nat_tiles[(b, st)] = nat_t
                if (b, st) == (0, 0):
                    # split the pipeline-fill load so the first transposes
                    # can start after ~a quarter tile
                    for q in range(SQ):
                        src = hidden[b, st * 512 + q * 128:
                                     st * 512 + (q + 1) * 128, :]
                        nc.sync.dma_start(out=nat_t[:, q, :], in_=src)
                else:
                    src = hidden[b, st * 512:(st + 1) * 512, :].rearrange(
                        "(q p) h -> p q h", p=128
                    )
                    nc.sync.dma_start(out=nat_t[:], in_=src)
                issued.append((b, st))

            def ensure_loads(upto):
                for gi in range(len(issued), min(upto + 1, BPC * ST)):
                    load_tile(gi // ST, gi % ST)

            def emit_Tblock(b, st):
                """Produce the 4 hT tiles (8 h-chunks) for s-tile st."""
                ensure_loads(b * ST + st + LOOKAHEAD)
                nat_t = nat_tiles[(b, st)]
                hTs = []
                for jp in range(4):
                    hT = hT_pool.tile([128, 1024], F16, tag="hT")
                    hTs.append(hT)
                    if (b, st, jp) in XBAR_JPS:
                        for ji in range(2):
                            j = jp * 2 + ji
                            nc.sync.dma_start_transpose(
                                hT[:, ji * 512:(ji + 1) * 512],
                                hidden[b, st * 512:(st + 1) * 512,
                                       j * 128:(j + 1) * 128],
                            )
                        continue
                    ps_t = psT_pool.tile([128, 1024], F16, tag="psT")
                    for ji in range(2):
                        j = jp * 2 + ji
                        for q in range(SQ):
                            # two concurrent half-width transposes per block
                            dst = ps_t[:, ji * 512 + q * 128:
                                       ji * 512 + (q + 1) * 128]
                            src = nat_t[:, q, j * 128:(j + 1) * 128]
                            nc.tensor.transpose(
                                dst[0:64, :], src[:, 0:64], ident_sb[:],
                                tile_position=(0, 0),
                            )
                            nc.tensor.transpose(
                                dst[64:128, :], src[:, 64:128], ident_sb[:],
                                tile_position=(0, 64),
                            )
                    if jp % 2 == 0:
                        nc.scalar.copy(hT[:], ps_t[:])
                    else:
                        nc.vector.tensor_copy(hT[:], ps_t[:])
                hT_sets[(b, st)] = hTs

            def emit_MM_pair(b, p):
                """Paired scores matmuls for s-tiles (2p, 2p+1) plus the
                per-tile max; scores stay in PSUM until the half max is
                known (deferred exp)."""
                t0, t1 = 2 * p, 2 * p + 1
                stats = st_stats[b]
                hTs0 = hT_sets.pop((b, t0))
                hTs1 = hT_sets.pop((b, t1))
                ps_sc = psS_pool.tile([128, 512], F32, tag="psS",
                                      name=f"psS{b}_{p}")
                ps_pair[(b, p)] = ps_sc
                for j in range(HJ):
                    jp, ji = j // 2, j % 2
                    nc.tensor.matmul(
                        ps_sc[0:64, :],
                        qT_sb[:, j, 0:64],
                        hTs0[jp][:, ji * 512:(ji + 1) * 512],
                        start=(j == 0),
                        stop=(j == HJ - 1),
                        tile_position=(0, 0),
                        skip_group_check=True,
                    )
                    nc.tensor.matmul(
                        ps_sc[64:128, :],
                        qT_sb[:, j, 64:128],
                        hTs1[jp][:, ji * 512:(ji + 1) * 512],
                        start=(j == 0),
                        stop=(j == HJ - 1),
                        tile_position=(0, 64),
                        skip_group_check=True,
                    )
                for st, base in ((t0, 0), (t1, 64)):
                    nc.vector.reduce_max(
                        stats[:, NEGM + st:NEGM + st + 1],
                        ps_sc[base:base + 64, :],
                        axis=mybir.AxisListType.X, negate=True,
                    )

            def combine_half(b, half, t0, cnt):
                """negM over the half's tiles."""
                stats = st_stats[b]
                nc.vector.tensor_reduce(
                    stats[:, NEGM_H + half:NEGM_H + half + 1],
                    stats[:, NEGM + t0:NEGM + t0 + cnt],
                    axis=mybir.AxisListType.X, op=mybir.AluOpType.min,
                )

            fT_sets = {}

            def phase3_T(b, st, half):
                """Deferred exp (bias = -M_half) + factor transposes."""
                stats = st_stats[b]
                exp_sb = exp_tiles[b]
                ps_sc = ps_pair[(b, st // 2)]
                base = 64 * (st % 2)
                nc.scalar.activation(
                    exp_sb[:, st * 512:(st + 1) * 512],
                    ps_sc[base:base + 64, :],
                    mybir.ActivationFunctionType.Exp,
                    bias=stats[:, NEGM_H + half:NEGM_H + half + 1],
                    accum_out=stats[:, RS + st:RS + st + 1],
                )
                ps_f = psT_pool.tile([128, 1024], F16, tag="psT")
                for q in range(SQ):
                    k = st * SQ + q
                    src = exp_sb[:, k * 128:(k + 1) * 128]
                    nc.tensor.transpose(
                        ps_f[0:64, q * C:(q + 1) * C],
                        src[:, 0:64], ident_sb[:C, :C],
                        tile_position=(0, 0),
                    )
                    nc.tensor.transpose(
                        ps_f[64:128, q * C:(q + 1) * C],
                        src[:, 64:128], ident_sb[:C, :C],
                        tile_position=(0, 64),
                    )
                fT = fT_pool.tile([128, SQ * C], F16, tag="fT")
                if st % 2 == 0:
                    nc.scalar.copy(fT[:], ps_f[:, :SQ * C])
                else:
                    nc.vector.tensor_copy(fT[:], ps_f[:, :SQ * C])
                fT_sets[(b, st)] = fT

            def phase3_MM(b, st, half, first, last):
                """Pooling matmuls, column-tiled over h-halves."""
                ps_res = ps_half[(b, half)]
                fT = fT_sets.pop((b, st))
                nat_t = nat_tiles[(b, st)]
                for q in range(SQ):
                    w = fT[:, q * C:(q + 1) * C]
                    nc.tensor.matmul(
                        ps_res[0:64, :],
                        w,
                        nat_t[:, q, 0:512],
                        start=(first and q == 0),
                        stop=(last and q == SQ - 1),
                        tile_position=(0, 0),
                        skip_group_check=True,
                    )
                    nc.tensor.matmul(
                        ps_res[64:128, :],
                        w,
                        nat_t[:, q, 512:1024],
                        start=(first and q == 0),
                        stop=(last and q == SQ - 1),
                        tile_position=(0, 64),
                        skip_group_check=True,
                    )

            def finalize(b):
                """Combine the two half accumulators and write out."""
                stats = st_stats[b]
                negM_G = stats_pool.tile([C, 1], F32, tag="negMG")
                nc.vector.tensor_scalar_min(
                    negM_G[:], stats[:, NEGM_H:NEGM_H + 1],
                    stats[:, NEGM_H + 1:NEGM_H + 2],
                )
                # gamma_half = exp(M_half - M) = exp(-negM_half + negM)
                nc.scalar.activation(
                    stats[:, GAM_H:GAM_H + 2],
                    stats[:, NEGM_H:NEGM_H + 2],
                    mybir.ActivationFunctionType.Exp,
                    bias=negM_G[:],
                    scale=-1.0,
                )
                # S_half = sum of the half's per-tile rowsums
                nA = 4
                sg = stats_pool.tile([C, 4], F32, tag="sg")
                nc.vector.reduce_sum(
                    stats[:, SH:SH + 1], stats[:, RS:RS + nA],
                    axis=mybir.AxisListType.X,
                )
                nc.vector.reduce_sum(
                    stats[:, SH + 1:SH + 2], stats[:, RS + nA:RS + ST],
                    axis=mybir.AxisListType.X,
                )
                nc.vector.tensor_scalar_mul(
                    sg[:, 0:1], stats[:, SH:SH + 1], stats[:, GAM_H:GAM_H + 1],
                )
                nc.vector.tensor_scalar_mul(
                    sg[:, 1:2], stats[:, SH + 1:SH + 2],
                    stats[:, GAM_H + 1:GAM_H + 2],
                )
                nc.vector.tensor_add(sg[:, 2:3], sg[:, 0:1], sg[:, 1:2])
                rinv = stats_pool.tile([C, 1], F32, tag="rinv")
                nc.vector.reciprocal(rinv[:], sg[:, 2:3])
                alph = stats_pool.tile([C, 2], F32, tag="alph")
                nc.vector.tensor_scalar_mul(
                    alph[:], stats[:, GAM_H:GAM_H + 2], rinv[:],
                )
                # out[:, h-half] = alphA*psA[rows] + alphB*psB[rows]
                psA = ps_half[(b, 0)]
                psB = ps_half[(b, 1)]
                out_sb = out_pool.tile([C, H], F32, tag="out")
                for h2, base in ((0, 0), (1, 64)):
                    u = out_pool.tile([C, 512], F32, tag=f"u{h2}")
                    nc.vector.tensor_scalar_mul(
                        u[:], psA[base:base + 64, :], alph[:, 0:1],
                    )
                    nc.vector.scalar_tensor_tensor(
                        out_sb[:, h2 * 512:(h2 + 1) * 512],
                        psB[base:base + 64, :],
                        alph[:, 1:2],
                        u[:],
                        op0=mybir.AluOpType.mult,
                        op1=mybir.AluOpType.add,
                    )
                nc.sync.dma_start(out=out[b], in_=out_sb[:])

            # ---- schedule ----
            for b in range(BPC):
                exp_tiles[b] = exp_pool.tile([C, S], F16, tag="expf",
                                             name=f"expf{b}")
                st_stats[b] = stats_pool.tile([C, 32], F32, tag="stats",
                                              name=f"stats{b}")
            ensure_loads(LOOKAHEAD)

            pending = []   # deferred phase3/finalize closures

            def pop(n=1):
                for _ in range(n):
                    if pending:
                        pending.pop(0)()

            def emit_half(b, half, p0, pcnt, drain_extra=False):
                pairs = list(range(p0, p0 + pcnt))
                tiles = list(range(2 * p0, 2 * (p0 + pcnt)))
                emit_Tblock(b, tiles[0])
                emit_Tblock(b, tiles[1])
                for i, p in enumerate(pairs):
                    if i + 1 < len(pairs):
                        emit_Tblock(b, 2 * pairs[i + 1])
                        emit_Tblock(b, 2 * pairs[i + 1] + 1)
                    emit_MM_pair(b, p)
                    if i > 0 or half == 1 or b > 0:
                        pop(3 if drain_extra else 2)
                combine_half(b, half, tiles[0], len(tiles))
                ps_half[(b, half)] = psR_pool.tile([128, 512], F32, tag="psR",
                                                   name=f"psR{b}_{half}")
                # phase3 items pipelined one-behind: item i emits tile i's
                # exp + transposes plus tile i-1's pooling matmuls, so the
                # fT evacuation copy is never on the PE critical path
                for i, st in enumerate(tiles):
                    def p3(b=b, st=st, half=half, i=i, tiles=tiles):
                        phase3_T(b, st, half)
                        if i > 0:
                            phase3_MM(b, tiles[i - 1], half,
                                      first=(i - 1 == 0), last=False)
                    pending.append(p3)

                def p3_flush(b=b, half=half, st=tiles[-1], cnt=len(tiles)):
                    phase3_MM(b, st, half,
                              first=(cnt == 1), last=True)
                pending.append(p3_flush)

            for b in range(BPC):
                emit_half(b, 0, 0, NPAIR // 2)
                emit_half(b, 1, NPAIR // 2, NPAIR - NPAIR // 2,
                          drain_extra=(b == BPC - 1))
                pending.append(lambda b=b: finalize(b))
            pop(len(pending))

    nc.compile()
    return nc


_NC_CACHE = None


def _get_nc():
    global _NC_CACHE
    if _NC_CACHE is None:
        _NC_CACHE = build_nc()
    return _NC_CACHE


def kernel(hidden, querys):
    hidden = np.asarray(hidden)
    querys = np.asarray(querys, dtype=np.float32)
    assert hidden.shape == (B, S, H) and querys.shape == (C, H)

    hidden16 = np.ascontiguousarray(hidden, dtype=np.float16)

    # qT[k, j, c] = querys[c, j*128 + k], duplicated into both column groups
    qT = np.ascontiguousarray(
        querys.T.reshape(HJ, 128, C).transpose(1, 0, 2)
    ).astype(np.float16)
    qT2 = np.concatenate([qT, qT], axis=2)          # [128, HJ, 128]
    ident = np.eye(128, dtype=np.float16)

    nc = _get_nc()
    in_maps = [
        {
            "hidden": np.ascontiguousarray(hidden16[i * BPC:(i + 1) * BPC]),
            "qT2": qT2,
            "ident": ident,
        }
        for i in range(NCORES)
    ]
    res = run_bass_kernel_spmd(nc, in_maps, core_ids=list(range(NCORES)))
    global LAST_RESULTS
    LAST_RESULTS = res
    outs = [np.asarray(res.results[i]["out"]).reshape(BPC, C * H)
            for i in range(NCORES)]
    return np.concatenate(outs, axis=0)


LAST_RESULTS = None


# revision 5
# speedup vs baseline: 2.9896x; 2.9896x over previous
"""Trainium2 Bass kernel for AttentionFact:
    scores = einsum('bsh,ch->bcs', hidden, querys)
    factor = softmax(scores, axis=2)
    out    = einsum('bcs,bsh->bch', factor, hidden).reshape(B, C*H)

Shapes: B=16, S=4096, H=1024, C=64, fp32.

Strategy (v5): data-parallel over batch, 2 batches per core, querys
replicated.  Host pre-casts hidden to fp16 (halves HBM traffic) and
pre-transposes querys into a duplicated [128, 8, 128] bank.

PE work is column-tiled throughout (tile_position (0,0)/(0,64)):
  - hT transposes: each 128x128 block is split into two concurrent
    64-col transposes -> ~2x faster transpose pass.
  - scores: s-tiles processed in PAIRS; member 0 accumulates into
    PSUM rows 0:64, member 1 into rows 64:128 (duplicated qT weights).
  - pooling: column groups split H (h 0:512 rows 0:64, h 512:1024
    rows 64:128) with identical fT weights.

Softmax: flash-style over two per-batch halves, with DEFERRED exp:
scores stay in PSUM until the half's max is known, then exp uses
bias = -M_half directly (no per-tile beta fold pass).

Optionally some hT groups are produced by DRAM-direct xbar DMA
transpose instead of the PE (XBAR_JPS).
"""

import numpy as np

import concourse.bass as bass
import concourse.mybir as mybir
import concourse.tile as tile
from concourse import bacc
from concourse.bass_utils import run_bass_kernel_spmd

B, S, H, C = 16, 4096, 1024, 64
NCORES = 8
BPC = B // NCORES          # batches per core
ST = 8                     # s-tiles per batch (512 rows each)
SQ = 4                     # 128-row subtiles per s-tile
HJ = H // 128              # h-chunks (8)
NPAIR = ST // 2            # s-tile pairs per batch
LOOKAHEAD = 4              # tile loads kept in flight ahead of compute

F32 = mybir.dt.float32
F16 = mybir.dt.float16

# (b, st, jp) triples whose hT group (2 chunks) comes from DRAM via the
# xbar DMA transpose instead of PE transposes (jp in 0..3)
XBAR_JPS = set()


def build_nc():
    nc = bacc.Bacc("TRN2", target_bir_lowering=False, debug=False)
    hidden = nc.declare_dram_parameter("hidden", [BPC, S, H], F16, isOutput=False)
    # qT2[k, j, c] = qT2[k, j, 64+c] = querys[c, j*128+k]
    qT2 = nc.declare_dram_parameter("qT2", [128, HJ, 128], F16, isOutput=False)
    ident = nc.declare_dram_parameter("ident", [128, 128], F16, isOutput=False)
    out = nc.declare_dram_parameter("out", [BPC, C, H], F32, isOutput=True)

    with tile.TileContext(nc) as tc:
        with (
            tc.tile_pool(name="const", bufs=1) as const_pool,
            tc.tile_pool(name="nat", bufs=2 * ST) as nat_pool,
            tc.tile_pool(name="hT", bufs=12) as hT_pool,
            tc.tile_pool(name="expp", bufs=2) as exp_pool,
            tc.tile_pool(name="fT", bufs=2) as fT_pool,
            tc.tile_pool(name="stats", bufs=2) as stats_pool,
            tc.tile_pool(name="outp", bufs=2) as out_pool,
            tc.tile_pool(name="psT", bufs=2, space="PSUM") as psT_pool,
            tc.tile_pool(name="psS", bufs=4, space="PSUM") as psS_pool,
            tc.tile_pool(name="psR", bufs=2, space="PSUM") as psR_pool,
        ):
            ident_sb = const_pool.tile([128, 128], F16, tag="ident")
            nc.sync.dma_start(out=ident_sb[:], in_=ident[:])
            qT_sb = const_pool.tile([128, HJ, 128], F16, tag="qT2")
            nc.sync.dma_start(out=qT_sb[:], in_=qT2[:])

            nat_tiles = {}
            hT_sets = {}
            ps_pair = {}    # pair p -> psS tile holding scores of (2p, 2p+1)
            exp_tiles = {}
            st_stats = {}   # b -> stats tile [64, 32] f32 laid out below
            ps_half = {}    # (b, half) -> psum accumulator [128, 512]

            # stats tile columns: 0:8 negm (-tile max), 8:16 rowsum,
            # 24:26 negM half, 26:28 gamma half, 28:30 S halves
            NEGM, RS, NEGM_H, GAM_H, SH = 0, 8, 24, 26, 28

            issued = []

            def load_tile(b, st):
                nat_t = nat_pool.tile([128, SQ, H], F16, tag="nat",
                                      name=f"nat{b}_{st}")
                nat_tiles[(b, st)] = nat_t
                if (b, st) == (0, 0):
                    # split the pipeline-fill load so the first transposes
                    # can start after ~a quarter tile
                    for q in range(SQ):
                        src = hidden[b, st * 512 + q * 128:
                                     st * 512 + (q + 1) * 128, :]
                        nc.sync.dma_start(out=nat_t[:, q, :], in_=src)
                else:
                    src = hidden[b, st * 512:(st + 1) * 512, :].rearrange(
                        "(q p) h -> p q h", p=128
                    )
                    nc.sync.dma_start(out=nat_t[:], in_=src)
                issued.append((b, st))

            def ensure_loads(upto):
                for gi in range(len(issued), min(upto + 1, BPC * ST)):
                    load_tile(gi // ST, gi % ST)

            def emit_Tblock(b, st):
                """Produce the 4 hT tiles (8 h-chunks) for s-tile st."""
                ensure_loads(b * ST + st + LOOKAHEAD)
                nat_t = nat_tiles[(b, st)]
                hTs = []
                for jp in range(4):
                    hT = hT_pool.tile([128, 1024], F16, tag="hT")
                    hTs.append(hT)
                    if (b, st, jp) in XBAR_JPS:
                        for ji in range(2):
                            j = jp * 2 + ji
                            nc.sync.dma_start_transpose(
                                hT[:, ji * 512:(ji + 1) * 512],
                                hidden[b, st * 512:(st + 1) * 512,
                                       j * 128:(j + 1) * 128],
                            )
                        continue
                    ps_t = psT_pool.tile([128, 1024], F16, tag="psT")
                    for ji in range(2):
                        j = jp * 2 + ji
                        for q in range(SQ):
                            nc.tensor.transpose(
                                ps_t[:, ji * 512 + q * 128:
                                     ji * 512 + (q + 1) * 128],
                                nat_t[:, q, j * 128:(j + 1) * 128],
                                ident_sb[:],
                            )
                    if jp % 2 == 0:
                        nc.scalar.copy(hT[:], ps_t[:])
                    else:
                        nc.vector.tensor_copy(hT[:], ps_t[:])
                hT_sets[(b, st)] = hTs

            def emit_MM_pair(b, p):
                """Paired scores matmuls for s-tiles (2p, 2p+1) plus the
                per-tile max; scores stay in PSUM until the half max is
                known (deferred exp)."""
                t0, t1 = 2 * p, 2 * p + 1
                stats = st_stats[b]
                hTs0 = hT_sets.pop((b, t0))
                hTs1 = hT_sets.pop((b, t1))
                ps_sc = psS_pool.tile([128, 512], F32, tag="psS",
                                      name=f"psS{b}_{p}")
                ps_pair[(b, p)] = ps_sc
                for j in range(HJ):
                    jp, ji = j // 2, j % 2
                    nc.tensor.matmul(
                        ps_sc[0:64, :],
                        qT_sb[:, j, 0:64],
                        hTs0[jp][:, ji * 512:(ji + 1) * 512],
                        start=(j == 0),
                        stop=(j == HJ - 1),
                        tile_position=(0, 0),
                        skip_group_check=True,
                    )
                    nc.tensor.matmul(
                        ps_sc[64:128, :],
                        qT_sb[:, j, 64:128],
                        hTs1[jp][:, ji * 512:(ji + 1) * 512],
                        start=(j == 0),
                        stop=(j == HJ - 1),
                        tile_position=(0, 64),
                        skip_group_check=True,
                    )
                for st, base in ((t0, 0), (t1, 64)):
                    nc.vector.reduce_max(
                        stats[:, NEGM + st:NEGM + st + 1],
                        ps_sc[base:base + 64, :],
                        axis=mybir.AxisListType.X, negate=True,
                    )

            def combine_half(b, half, t0, cnt):
                """negM over the half's tiles."""
                stats = st_stats[b]
                nc.vector.tensor_reduce(
                    stats[:, NEGM_H + half:NEGM_H + half + 1],
                    stats[:, NEGM + t0:NEGM + t0 + cnt],
                    axis=mybir.AxisListType.X, op=mybir.AluOpType.min,
                )

            fT_sets = {}

            def phase3_T(b, st, half):
                """Deferred exp (bias = -M_half) + factor transposes."""
                stats = st_stats[b]
                exp_sb = exp_tiles[b]
                ps_sc = ps_pair[(b, st // 2)]
                base = 64 * (st % 2)
                nc.scalar.activation(
                    exp_sb[:, st * 512:(st + 1) * 512],
                    ps_sc[base:base + 64, :],
                    mybir.ActivationFunctionType.Exp,
                    bias=stats[:, NEGM_H + half:NEGM_H + half + 1],
                    accum_out=stats[:, RS + st:RS + st + 1],
                )
                ps_f = psT_pool.tile([128, 1024], F16, tag="psT")
                for q in range(SQ):
                    k = st * SQ + q
                    nc.tensor.transpose(
                        ps_f[:, q * C:(q + 1) * C],
                        exp_sb[:, k * 128:(k + 1) * 128],
                        ident_sb[:C, :C],
                    )
                fT = fT_pool.tile([128, SQ * C], F16, tag="fT")
                if st % 2 == 0:
                    nc.scalar.copy(fT[:], ps_f[:, :SQ * C])
                else:
                    nc.vector.tensor_copy(fT[:], ps_f[:, :SQ * C])
                fT_sets[(b, st)] = fT

            def phase3_MM(b, st, half, first, last):
                """Pooling matmuls, column-tiled over h-halves."""
                ps_res = ps_half[(b, half)]
                fT = fT_sets.pop((b, st))
                nat_t = nat_tiles[(b, st)]
                for q in range(SQ):
                    w = fT[:, q * C:(q + 1) * C]
                    nc.tensor.matmul(
                        ps_res[0:64, :],
                        w,
                        nat_t[:, q, 0:512],
                        start=(first and q == 0),
                        stop=(last and q == SQ - 1),
                        tile_position=(0, 0),
                        skip_group_check=True,
                    )
                    nc.tensor.matmul(
                        ps_res[64:128, :],
                        w,
                        nat_t[:, q, 512:1024],
                        start=(first and q == 0),
                        stop=(last and q == SQ - 1),
                        tile_position=(0, 64),
                        skip_group_check=True,
                    )

            def finalize(b):
                """Combine the two half accumulators and write out."""
                stats = st_stats[b]
                negM_G = stats_pool.tile([C, 1], F32, tag="negMG")
                nc.vector.tensor_scalar_min(
                    negM_G[:], stats[:, NEGM_H:NEGM_H + 1],
                    stats[:, NEGM_H + 1:NEGM_H + 2],
                )
                # gamma_half = exp(M_half - M) = exp(-negM_half + negM)
                nc.scalar.activation(
                    stats[:, GAM_H:GAM_H + 2],
                    stats[:, NEGM_H:NEGM_H + 2],
                    mybir.ActivationFunctionType.Exp,
                    bias=negM_G[:],
                    scale=-1.0,
                )
                # S_half = sum of the half's per-tile rowsums
                nA = 4
                sg = stats_pool.tile([C, 4], F32, tag="sg")
                nc.vector.reduce_sum(
                    stats[:, SH:SH + 1], stats[:, RS:RS + nA],
                    axis=mybir.AxisListType.X,
                )
                nc.vector.reduce_sum(
                    stats[:, SH + 1:SH + 2], stats[:, RS + nA:RS + ST],
                    axis=mybir.AxisListType.X,
                )
                nc.vector.tensor_scalar_mul(
                    sg[:, 0:1], stats[:, SH:SH + 1], stats[:, GAM_H:GAM_H + 1],
                )
                nc.vector.tensor_scalar_mul(
                    sg[:, 1:2], stats[:, SH + 1:SH + 2],
                    stats[:, GAM_H + 1:GAM_H + 2],
                )
                nc.vector.tensor_add(sg[:, 2:3], sg[:, 0:1], sg[:, 1:2])
                rinv = stats_pool.tile([C, 1], F32, tag="rinv")
                nc.vector.reciprocal(rinv[:], sg[:, 2:3])
                alph = stats_pool.tile([C, 2], F32, tag="alph")
                nc.vector.tensor_scalar_mul(
                    alph[:], stats[:, GAM_H:GAM_H + 2], rinv[:],
                )
                # out[:, h-half] = alphA*psA[rows] + alphB*psB[rows]
                psA = ps_half[(b, 0)]
                psB = ps_half[(b, 1)]
                out_sb = out_pool.tile([C, H], F32, tag="out")
                for h2, base in ((0, 0), (1, 64)):
                    u = out_pool.tile([C, 512], F32, tag=f"u{h2}")
                    nc.vector.tensor_scalar_mul(
                        u[:], psA[base:base + 64, :], alph[:, 0:1],
                    )
                    nc.vector.scalar_tensor_tensor(
                        out_sb[:, h2 * 512:(h2 + 1) * 512],
                        psB[base:base + 64, :],
                        alph[:, 1:2],
                        u[:],
                        op0=mybir.AluOpType.mult,
                        op1=mybir.AluOpType.add,
                    )
                nc.sync.dma_start(out=out[b], in_=out_sb[:])

            # ---- schedule ----
            for b in range(BPC):
                exp_tiles[b] = exp_pool.tile([C, S], F16, tag="expf",
                                             name=f"expf{b}")
                st_stats[b] = stats_pool.tile([C, 32], F32, tag="stats",
                                              name=f"stats{b}")
            ensure_loads(LOOKAHEAD)

            pending = []   # deferred phase3/finalize closures

            def pop(n=1):
                for _ in range(n):
                    if pending:
                        pending.pop(0)()

            def emit_half(b, half, p0, pcnt, drain_extra=False):
                pairs = list(range(p0, p0 + pcnt))
                tiles = list(range(2 * p0, 2 * (p0 + pcnt)))
                emit_Tblock(b, tiles[0])
                emit_Tblock(b, tiles[1])
                for i, p in enumerate(pairs):
                    if i + 1 < len(pairs):
                        emit_Tblock(b, 2 * pairs[i + 1])
                        emit_Tblock(b, 2 * pairs[i + 1] + 1)
                    emit_MM_pair(b, p)
                    if i > 0 or half == 1 or b > 0:
                        pop(3 if drain_extra else 2)
                combine_half(b, half, tiles[0], len(tiles))
                ps_half[(b, half)] = psR_pool.tile([128, 512], F32, tag="psR",
                                                   name=f"psR{b}_{half}")
                # phase3 items pipelined one-behind: item i emits tile i's
                # exp + transposes plus tile i-1's pooling matmuls, so the
                # fT evacuation copy is never on the PE critical path
                for i, st in enumerate(tiles):
                    def p3(b=b, st=st, half=half, i=i, tiles=tiles):
                        phase3_T(b, st, half)
                        if i > 0:
                            phase3_MM(b, tiles[i - 1], half,
                                      first=(i - 1 == 0), last=False)
                    pending.append(p3)

                def p3_flush(b=b, half=half, st=tiles[-1], cnt=len(tiles)):
                    phase3_MM(b, st, half,
                              first=(cnt == 1), last=True)
                pending.append(p3_flush)

            for b in range(BPC):
                emit_half(b, 0, 0, NPAIR // 2)
                emit_half(b, 1, NPAIR // 2, NPAIR - NPAIR // 2,
                          drain_extra=(b == BPC - 1))
                pending.append(lambda b=b: finalize(b))
            pop(len(pending))

    nc.compile()
    return nc


_NC_CACHE = None


def _get_nc():
    global _NC_CACHE
    if _NC_CACHE is None:
        _NC_CACHE = build_nc()
    return _NC_CACHE


def kernel(hidden, querys):
    hidden = np.asarray(hidden)
    querys = np.asarray(querys, dtype=np.float32)
    assert hidden.shape == (B, S, H) and querys.shape == (C, H)

    hidden16 = np.ascontiguousarray(hidden, dtype=np.float16)

    # qT[k, j, c] = querys[c, j*128 + k], duplicated into both column groups
    qT = np.ascontiguousarray(
        querys.T.reshape(HJ, 128, C).transpose(1, 0, 2)
    ).astype(np.float16)
    qT2 = np.concatenate([qT, qT], axis=2)          # [128, HJ, 128]
    ident = np.eye(128, dtype=np.float16)

    nc = _get_nc()
    in_maps = [
        {
            "hidden": np.ascontiguousarray(hidden16[i * BPC:(i + 1) * BPC]),
            "qT2": qT2,
            "ident": ident,
        }
        for i in range(NCORES)
    ]
    res = run_bass_kernel_spmd(nc, in_maps, core_ids=list(range(NCORES)))
    global LAST_RESULTS
    LAST_RESULTS = res
    outs = [np.asarray(res.results[i]["out"]).reshape(BPC, C * H)
            for i in range(NCORES)]
    return np.concatenate(outs, axis=0)


LAST_RESULTS = None
